# revision 16
# baseline (speedup 1.0000x reference)
"""Trainium2 Bass kernel for nn_MultiHeadAttention_63986422775834.

Computation (see harness reference):
    q = x @ Wq + bq; k = x @ Wk + bk; v = x @ Wv + bv          # [N, D]
    group rows by 8: scores[b,h,g] = q[8b+h] . k[8b+g] / sqrt(D)
    w = softmax(scores, axis=-1);  out[8b+h] = sum_g w[b,h,g] * v[8b+g]

Sharding: data-parallel over rows across 8 NeuronCores (2048 rows each;
row groups of 8 never cross a shard boundary). Weights replicated.

Per-core kernel (bf16 matmuls, fp32 accumulate):
  phase A: load x strips, cast bf16, PE-transpose -> resident xT tiles
           (d_in on partitions).
  pass 1:  stream Wq/Wk in d_out chunks; qT/kT = W.T-oriented projection
           GEMMs (d_out on partitions); S[128x128 row-block diag tiles]
           accumulated in SBUF over d_out chunks; masked softmax over
           8x8 diagonal blocks; PE-transpose the softmax weights.
  pass 2:  stream Wv; V tiles (rows on partitions); O = w @ V + bv; DMA out.
"""

import sys

sys.path.insert(0, "/opt/trn_rl_repo")

import numpy as np
import ml_dtypes

import concourse.bass as bass
import concourse.mybir as mybir
import concourse.tile as tile
from concourse import bacc
from concourse.bass_utils import run_bass_kernel_spmd

# problem shape (hardcoded per contract)
N_FULL = 16384
D = 2048
H = 8
N_CORES = 8
R = N_FULL // N_CORES  # rows per core = 2048
P = 128
KO = D // P  # 16 k-subtiles along d_in
NB = 4  # row blocks per core
RB = R // NB  # 512 rows per block
NSUB = RB // P  # 4 row subtiles per block
SCALE = 1.0 / np.sqrt(np.float32(D))

BF16 = mybir.dt.bfloat16
F32 = mybir.dt.float32

# pass-1 d_out chunking: 8 chunks x 256 (2 j-tiles of 128)
NC1 = 8
C1 = D // NC1  # 256
JJ1 = C1 // P  # 2
# pass-2 d_out chunking: 8 chunks x 256
NC2 = 8
C2 = D // NC2  # 256


def build_program():
    nc = bacc.Bacc("TRN2", target_bir_lowering=False, debug=False, num_devices=N_CORES)

    xs = nc.dram_tensor("xs", [R, D], F32, kind="ExternalInput")
    Wq = nc.dram_tensor("Wq", [D, D], F32, kind="ExternalInput")
    Wk = nc.dram_tensor("Wk", [D, D], F32, kind="ExternalInput")
    Wv = nc.dram_tensor("Wv", [D, D], F32, kind="ExternalInput")
    bqt = nc.dram_tensor("bqt", [P, KO], F32, kind="ExternalInput")
    bkt = nc.dram_tensor("bkt", [P, KO], F32, kind="ExternalInput")
    bvr = nc.dram_tensor("bvr", [P, D], F32, kind="ExternalInput")
    maskt = nc.dram_tensor("maskt", [P, P], F32, kind="ExternalInput")
    ident = nc.dram_tensor("ident", [P, P], BF16, kind="ExternalInput")
    out = nc.dram_tensor("out", [R, D], F32, kind="ExternalOutput")

    # d_in-major views of the weights: w[p, ko, n] = W[ko*128+p, n]
    wq_ap = Wq[:].rearrange("(ko p) n -> p ko n", p=P)
    wk_ap = Wk[:].rearrange("(ko p) n -> p ko n", p=P)
    wv_ap = Wv[:].rearrange("(ko p) n -> p ko n", p=P)

    with tile.TileContext(nc) as tc:
        with (
            tc.tile_pool(name="const", bufs=1) as const,
            tc.tile_pool(name="xT", bufs=1) as xT_pool,
            tc.tile_pool(name="phA", bufs=2) as phA,
            tc.tile_pool(name="wchunk", bufs=2) as wchunk,
            tc.tile_pool(name="wtmp", bufs=2) as wtmp,
            tc.tile_pool(name="qk", bufs=8) as qkp,
            tc.tile_pool(name="sacc", bufs=1) as sacc,
            tc.tile_pool(name="soft", bufs=2) as soft,
            tc.tile_pool(name="vpool", bufs=3) as vpool,
            tc.tile_pool(name="opool", bufs=2) as opool,
            tc.tile_pool(name="ps_big", bufs=3, space="PSUM") as ps_big,
            tc.tile_pool(name="ps_s", bufs=2, space="PSUM") as ps_s,
            tc.tile_pool(name="ps_t", bufs=2, space="PSUM") as ps_t,
            tc.tile_pool(name="dram", bufs=1, space="DRAM") as dram,
        ):
            # --- constants ---
            mask_sb = const.tile([P, P], F32)
            nc.sync.dma_start(mask_sb, maskt[:])
            ident_sb = const.tile([P, P], BF16)
            nc.sync.dma_start(ident_sb, ident[:])
            bq_sb = const.tile([P, KO], F32)
            nc.sync.dma_start(bq_sb, bqt[:])
            bk_sb = const.tile([P, KO], F32)
            nc.sync.dma_start(bk_sb, bkt[:])
            bv_sb = const.tile([P, D], F32)
            nc.sync.dma_start(bv_sb, bvr[:])

            # persistent intermediates
            # xT[b][p, ko, r] = x[b*RB + r, ko*128 + p]  (bf16)
            xT = [xT_pool.tile([P, KO, RB], BF16, name=f"xT{b}") for b in range(NB)]
            # S accumulator: S_all[p, i, :] for i = b*NSUB+sub, [r_h, r_g] f32
            S_all = sacc.tile([P, NB * NSUB, P], F32)
            # transposed softmax weights (lhsT for the O matmul)
            wT_all = sacc.tile([P, NB * NSUB, P], BF16)

            def load_w_chunk(w_ap, c, width, tag):
                dst = wchunk.tile([P, KO, width], BF16, tag=tag)
                for jj in range(width // P):
                    j = (c * width) // P + jj
                    tmp = wtmp.tile([P, KO, P], F32, tag="wtmp")
                    nc.sync.dma_start(tmp, w_ap[:, :, j * P : (j + 1) * P])
                    nc.vector.tensor_copy(dst[:, :, jj * P : (jj + 1) * P], tmp)
                return dst

            # Hoist the first W chunk loads so their DMAs start immediately.
            wq_tiles = {0: load_w_chunk(wq_ap, 0, C1, "wq")}
            wk_tiles = {0: load_w_chunk(wk_ap, 0, C1, "wk")}
            wv_tiles = {}

            # --- phase A: x -> bf16, PE-transpose into xT ---
            # Emission order sets DMA-ring order: interleave the pass-1 W
            # prefetches between phase-A blocks so neither starves the other.
            def phase_a_block(b):
                for rt in range(NSUB):  # 128-row strips
                    r0 = b * RB + rt * P
                    for s in range(2):  # 1024-col halves of d_in
                        xt = phA.tile([P, D // 2], F32, tag="xt")
                        nc.sync.dma_start(
                            xt, xs[r0 : r0 + P, s * (D // 2) : (s + 1) * (D // 2)]
                        )
                        xb = phA.tile([P, D // 2], BF16, tag="xb")
                        nc.vector.tensor_copy(xb, xt)
                        for t in range(KO // 2):  # 128-col tiles -> transpose
                            kt = s * (KO // 2) + t
                            pst = ps_t.tile([P, P], BF16, tag="tr")
                            nc.tensor.transpose(pst, xb[:, t * P : (t + 1) * P], ident_sb)
                            nc.vector.tensor_copy(
                                xT[b][:, kt, rt * P : (rt + 1) * P], pst
                            )

            phase_a_block(0)
            wq_tiles[1] = load_w_chunk(wq_ap, 1, C1, "wq")
            wk_tiles[1] = load_w_chunk(wk_ap, 1, C1, "wk")
            phase_a_block(1)
            phase_a_block(2)
            wv_tiles[0] = load_w_chunk(wv_ap, 0, C2, "wv")
            phase_a_block(3)

            # --- pass 1: qT/kT GEMMs + S accumulation + softmax ---
            pending_s = None  # (c, b, qts, kts) awaiting S matmuls

            def emit_s(c, b, qts, kts):
                for sub in range(NSUB):
                    pss = ps_s.tile([P, P], F32, tag="pss")
                    for jj in range(JJ1):
                        nc.tensor.matmul(
                            pss,
                            lhsT=qts[jj][:, sub * P : (sub + 1) * P],
                            rhs=kts[jj][:, sub * P : (sub + 1) * P],
                            start=(jj == 0),
                            stop=(jj == JJ1 - 1),
                        )
                    i = b * NSUB + sub
                    if c == 0:
                        nc.vector.tensor_copy(S_all[:, i, :], pss)
                    else:
                        nc.vector.tensor_add(S_all[:, i, :], S_all[:, i, :], pss)

            for c in range(NC1):
                # one-chunk emission lookahead keeps the next chunk's DMAs
                # ahead of this chunk's compute in the rings
                if c + 1 < NC1 and (c + 1) not in wq_tiles:
                    wq_tiles[c + 1] = load_w_chunk(wq_ap, c + 1, C1, "wq")
                    wk_tiles[c + 1] = load_w_chunk(wk_ap, c + 1, C1, "wk")
                wq_sb = wq_tiles.pop(c)
                wk_sb = wk_tiles.pop(c)
                for b in range(NB):
                    qts, kts = [], []
                    for jj in range(JJ1):
                        j = (c * C1) // P + jj
                        psq = ps_big.tile([P, RB], F32, tag="ps_big")
                        for kt in range(KO):
                            nc.tensor.matmul(
                                psq,
                                lhsT=wq_sb[:, kt, jj * P : (jj + 1) * P],
                                rhs=xT[b][:, kt, :],
                                start=(kt == 0),
                                stop=(kt == KO - 1),
                            )
                        qt = qkp.tile([P, RB], BF16, tag="qk")
                        nc.scalar.activation(
                            qt, psq, mybir.ActivationFunctionType.Identity,
                            bias=bq_sb[:, j : j + 1],
                        )
                        qts.append(qt)
                        psk = ps_big.tile([P, RB], F32, tag="ps_big")
                        for kt in range(KO):
                            nc.tensor.matmul(
                                psk,
                                lhsT=wk_sb[:, kt, jj * P : (jj + 1) * P],
                                rhs=xT[b][:, kt, :],
                                start=(kt == 0),
                                stop=(kt == KO - 1),
                            )
                        ktile = qkp.tile([P, RB], BF16, tag="qk")
                        nc.scalar.activation(
                            ktile, psk, mybir.ActivationFunctionType.Identity,
                            bias=bk_sb[:, j : j + 1],
                        )
                        kts.append(ktile)
                    if pending_s is not None:
                        emit_s(*pending_s)
                    pending_s = (c, b, qts, kts)
            if pending_s is not None:
                emit_s(*pending_s)
                pending_s = None

            # --- softmax + transpose of one weight tile ---
            def emit_softmax(i):
                tmask = soft.tile([P, P], F32, tag="tmask")
                nc.vector.tensor_add(tmask, S_all[:, i, :], mask_sb)
                e = soft.tile([P, P], F32, tag="e")
                ssum = soft.tile([P, 1], F32, tag="ssum")
                nc.scalar.activation(
                    e, tmask, mybir.ActivationFunctionType.Exp,
                    scale=float(SCALE), accum_out=ssum,
                )
                rcp = soft.tile([P, 1], F32, tag="rcp")
                nc.vector.reciprocal(rcp, ssum)
                wsb = soft.tile([P, P], BF16, tag="wsb")
                nc.vector.tensor_scalar_mul(wsb, e, rcp)
                pst = ps_t.tile([P, P], BF16, tag="tr")
                nc.tensor.transpose(pst, wsb, ident_sb)
                nc.vector.tensor_copy(wT_all[:, i, :], pst)

            # --- pass 2: V GEMM + O = w @ V + bv ---
            # softmax for tile i is interleaved after the c=0 V chain for i,
            # so the PE streams V matmuls while DVE/ACT run the softmax.
            pending_o = None  # (v_sb, b, rs, c)

            def emit_o(v_sb, b, rs, c):
                i = b * NSUB + rs
                pso = ps_big.tile([P, C2], F32, tag="ps_big")
                nc.tensor.matmul(
                    pso, lhsT=wT_all[:, i, :], rhs=v_sb, start=True, stop=True
                )
                o_sb = opool.tile([P, C2], F32, tag="o")
                nc.vector.tensor_add(o_sb, pso, bv_sb[:, c * C2 : (c + 1) * C2])
                r0 = b * RB + rs * P
                nc.sync.dma_start(out[r0 : r0 + P, c * C2 : (c + 1) * C2], o_sb)

            for c in range(NC2):
                if c + 1 < NC2 and (c + 1) not in wv_tiles:
                    wv_tiles[c + 1] = load_w_chunk(wv_ap, c + 1, C2, "wv")
                wv_sb = wv_tiles.pop(c)
                for b in range(NB):
                    for rs in range(NSUB):
                        psv = ps_big.tile([P, C2], F32, tag="ps_big")
                        for kt in range(KO):
                            nc.tensor.matmul(
                                psv,
                                lhsT=xT[b][:, kt, rs * P : (rs + 1) * P],
                                rhs=wv_sb[:, kt, :],
                                start=(kt == 0),
                                stop=(kt == KO - 1),
                            )
                        v_sb = vpool.tile([P, C2], BF16, tag="v")
                        nc.vector.tensor_copy(v_sb, psv)
                        if c == 0:
                            emit_softmax(b * NSUB + rs)
                        if pending_o is not None:
                            emit_o(*pending_o)
                        pending_o = (v_sb, b, rs, c)
            if pending_o is not None:
                emit_o(*pending_o)
                pending_o = None

    nc.compile()
    return nc


_CACHED = {}


def host_constants():
    mask = np.full((P, P), -1e9, dtype=np.float32)
    for g in range(P // H):
        mask[g * H : (g + 1) * H, g * H : (g + 1) * H] = 0.0
    identity = np.eye(P, dtype=ml_dtypes.bfloat16)
    return mask, identity


def kernel(x, Wq, bq, Wk, bk, Wv, bv):
    x = np.ascontiguousarray(np.asarray(x, dtype=np.float32))
    Wq = np.ascontiguousarray(np.asarray(Wq, dtype=np.float32))
    Wk = np.ascontiguousarray(np.asarray(Wk, dtype=np.float32))
    Wv = np.ascontiguousarray(np.asarray(Wv, dtype=np.float32))
    bq = np.asarray(bq, dtype=np.float32)
    bk = np.asarray(bk, dtype=np.float32)
    bv = np.asarray(bv, dtype=np.float32)

    if "nc" not in _CACHED:
        _CACHED["nc"] = build_program()
    nc = _CACHED["nc"]

    mask, identity = host_constants()
    bqt = np.ascontiguousarray(bq.reshape(KO, P).T)
    bkt = np.ascontiguousarray(bk.reshape(KO, P).T)
    bvr = np.ascontiguousarray(np.broadcast_to(bv, (P, D)))

    in_maps = []
    for i in range(N_CORES):
        in_maps.append(
            {
                "xs": x[i * R : (i + 1) * R],
                "Wq": Wq, "Wk": Wk, "Wv": Wv,
                "bqt": bqt, "bkt": bkt, "bvr": bvr,
                "maskt": mask, "ident": identity,
            }
        )
    res = run_bass_kernel_spmd(nc, in_maps, list(range(N_CORES)))
    return np.concatenate([res.results[i]["out"] for i in range(N_CORES)], axis=0)


# revision 17
# speedup vs baseline: 1.0530x; 1.0530x over previous
"""Trainium2 Bass kernel for nn_MultiHeadAttention_63986422775834.

Computation (see harness reference):
    q = x @ Wq + bq; k = x @ Wk + bk; v = x @ Wv + bv          # [N, D]
    group rows by 8: scores[b,h,g] = q[8b+h] . k[8b+g] / sqrt(D)
    w = softmax(scores, axis=-1);  out[8b+h] = sum_g w[b,h,g] * v[8b+g]

Sharding: data-parallel over rows across 8 NeuronCores (2048 rows each;
row groups of 8 never cross a shard boundary). Weights replicated.

Per-core kernel (bf16 matmuls, fp32 accumulate):
  phase A: load x strips, cast bf16, PE-transpose -> resident xT tiles
           (d_in on partitions).
  pass 1:  stream Wq/Wk in d_out chunks; qT/kT = W.T-oriented projection
           GEMMs (d_out on partitions); S[128x128 row-block diag tiles]
           accumulated in SBUF over d_out chunks; masked softmax over
           8x8 diagonal blocks; PE-transpose the softmax weights.
  pass 2:  stream Wv; V tiles (rows on partitions); O = w @ V + bv; DMA out.
"""

import sys

sys.path.insert(0, "/opt/trn_rl_repo")

import numpy as np
import ml_dtypes

import concourse.bass as bass
import concourse.mybir as mybir
import concourse.tile as tile
from concourse import bacc
from concourse.bass_utils import run_bass_kernel_spmd

# problem shape (hardcoded per contract)
N_FULL = 16384
D = 2048
H = 8
N_CORES = 8
R = N_FULL // N_CORES  # rows per core = 2048
P = 128
KO = D // P  # 16 k-subtiles along d_in
NB = 4  # row blocks per core
RB = R // NB  # 512 rows per block
NSUB = RB // P  # 4 row subtiles per block
SCALE = 1.0 / np.sqrt(np.float32(D))

BF16 = mybir.dt.bfloat16
F32 = mybir.dt.float32

# pass-1 d_out chunking: 8 chunks x 256 (2 j-tiles of 128)
NC1 = 8
C1 = D // NC1  # 256
JJ1 = C1 // P  # 2
# pass-2 d_out chunking: 4 chunks x 512
NC2 = 4
C2 = D // NC2  # 512


def build_program():
    nc = bacc.Bacc("TRN2", target_bir_lowering=False, debug=False, num_devices=N_CORES)

    xs = nc.dram_tensor("xs", [R, D], F32, kind="ExternalInput")
    Wq = nc.dram_tensor("Wq", [D, D], F32, kind="ExternalInput")
    Wk = nc.dram_tensor("Wk", [D, D], F32, kind="ExternalInput")
    Wv = nc.dram_tensor("Wv", [D, D], F32, kind="ExternalInput")
    bqt = nc.dram_tensor("bqt", [P, KO], F32, kind="ExternalInput")
    bkt = nc.dram_tensor("bkt", [P, KO], F32, kind="ExternalInput")
    bvr = nc.dram_tensor("bvr", [P, D], F32, kind="ExternalInput")
    maskt = nc.dram_tensor("maskt", [P, P], F32, kind="ExternalInput")
    ident = nc.dram_tensor("ident", [P, P], BF16, kind="ExternalInput")
    out = nc.dram_tensor("out", [R, D], F32, kind="ExternalOutput")

    # d_in-major views of the weights: w[p, ko, n] = W[ko*128+p, n]
    wq_ap = Wq[:].rearrange("(ko p) n -> p ko n", p=P)
    wk_ap = Wk[:].rearrange("(ko p) n -> p ko n", p=P)
    wv_ap = Wv[:].rearrange("(ko p) n -> p ko n", p=P)

    with tile.TileContext(nc) as tc:
        with (
            tc.tile_pool(name="const", bufs=1) as const,
            tc.tile_pool(name="xT", bufs=1) as xT_pool,
            tc.tile_pool(name="phA", bufs=4) as phA,
            tc.tile_pool(name="wchunk", bufs=2) as wchunk,
            tc.tile_pool(name="wtmp", bufs=2) as wtmp,
            tc.tile_pool(name="qk", bufs=8) as qkp,
            tc.tile_pool(name="sacc", bufs=1) as sacc,
            tc.tile_pool(name="soft", bufs=2) as soft,
            tc.tile_pool(name="vpool", bufs=3) as vpool,
            tc.tile_pool(name="opool", bufs=2) as opool,
            tc.tile_pool(name="ps_big", bufs=3, space="PSUM") as ps_big,
            tc.tile_pool(name="ps_s", bufs=2, space="PSUM") as ps_s,
            tc.tile_pool(name="ps_t", bufs=2, space="PSUM") as ps_t,
            tc.tile_pool(name="dram", bufs=1, space="DRAM") as dram,
        ):
            # --- constants ---
            mask_sb = const.tile([P, P], F32)
            nc.sync.dma_start(mask_sb, maskt[:])
            ident_sb = const.tile([P, P], BF16)
            nc.sync.dma_start(ident_sb, ident[:])
            bq_sb = const.tile([P, KO], F32)
            nc.sync.dma_start(bq_sb, bqt[:])
            bk_sb = const.tile([P, KO], F32)
            nc.sync.dma_start(bk_sb, bkt[:])
            bv_sb = const.tile([P, D], F32)
            nc.sync.dma_start(bv_sb, bvr[:])

            # persistent intermediates
            # xT[b][p, ko, r] = x[b*RB + r, ko*128 + p]  (bf16)
            xT = [xT_pool.tile([P, KO, RB], BF16, name=f"xT{b}") for b in range(NB)]
            # S accumulator: S_all[p, i, :] for i = b*NSUB+sub, [r_h, r_g] f32
            S_all = sacc.tile([P, NB * NSUB, P], F32)
            # transposed softmax weights (lhsT for the O matmul)
            wT_all = sacc.tile([P, NB * NSUB, P], BF16)

            def load_w_chunk(w_ap, c, width, tag):
                dst = wchunk.tile([P, KO, width], BF16, tag=tag)
                for jj in range(width // P):
                    j = (c * width) // P + jj
                    tmp = wtmp.tile([P, KO, P], F32, tag="wtmp")
                    nc.sync.dma_start(tmp, w_ap[:, :, j * P : (j + 1) * P])
                    nc.vector.tensor_copy(dst[:, :, jj * P : (jj + 1) * P], tmp)
                return dst

            # Hoist the first W chunk loads so their DMAs start immediately.
            wq_tiles = {0: load_w_chunk(wq_ap, 0, C1, "wq")}
            wk_tiles = {0: load_w_chunk(wk_ap, 0, C1, "wk")}
            wv_tiles = {}

            # --- phase A: x -> bf16, PE-transpose into xT ---
            # Emission order sets DMA-ring order: interleave the pass-1 W
            # prefetches between phase-A blocks so neither starves the other.
            def phase_a_block(b):
                for rt in range(NSUB):  # 128-row strips
                    r0 = b * RB + rt * P
                    for s in range(4):  # 512-col strips of d_in
                        xt = phA.tile([P, 512], F32, tag="xt")
                        nc.sync.dma_start(
                            xt, xs[r0 : r0 + P, s * 512 : (s + 1) * 512]
                        )
                        xb = phA.tile([P, 512], BF16, tag="xb")
                        nc.vector.tensor_copy(xb, xt)
                        for t in range(4):  # 128-col tiles -> transpose
                            kt = s * 4 + t
                            pst = ps_t.tile([P, P], BF16, tag="tr")
                            nc.tensor.transpose(pst, xb[:, t * P : (t + 1) * P], ident_sb)
                            nc.vector.tensor_copy(
                                xT[b][:, kt, rt * P : (rt + 1) * P], pst
                            )

            phase_a_block(0)
            wq_tiles[1] = load_w_chunk(wq_ap, 1, C1, "wq")
            wk_tiles[1] = load_w_chunk(wk_ap, 1, C1, "wk")
            phase_a_block(1)
            phase_a_block(2)
            wv_tiles[0] = load_w_chunk(wv_ap, 0, C2, "wv")
            phase_a_block(3)

            # --- pass 1: qT/kT GEMMs + S accumulation + softmax ---
            pending_s = None  # (c, b, qts, kts) awaiting S matmuls

            def emit_s(c, b, qts, kts):
                for sub in range(NSUB):
                    pss = ps_s.tile([P, P], F32, tag="pss")
                    for jj in range(JJ1):
                        nc.tensor.matmul(
                            pss,
                            lhsT=qts[jj][:, sub * P : (sub + 1) * P],
                            rhs=kts[jj][:, sub * P : (sub + 1) * P],
                            start=(jj == 0),
                            stop=(jj == JJ1 - 1),
                        )
                    i = b * NSUB + sub
                    if c == 0:
                        nc.vector.tensor_copy(S_all[:, i, :], pss)
                    else:
                        nc.vector.tensor_add(S_all[:, i, :], S_all[:, i, :], pss)

            for c in range(NC1):
                # one-chunk emission lookahead keeps the next chunk's DMAs
                # ahead of this chunk's compute in the rings
                if c + 1 < NC1 and (c + 1) not in wq_tiles:
                    wq_tiles[c + 1] = load_w_chunk(wq_ap, c + 1, C1, "wq")
                    wk_tiles[c + 1] = load_w_chunk(wk_ap, c + 1, C1, "wk")
                wq_sb = wq_tiles.pop(c)
                wk_sb = wk_tiles.pop(c)
                for b in range(NB):
                    qts, kts = [], []
                    for jj in range(JJ1):
                        j = (c * C1) // P + jj
                        psq = ps_big.tile([P, RB], F32, tag="ps_big")
                        for kt in range(KO):
                            nc.tensor.matmul(
                                psq,
                                lhsT=wq_sb[:, kt, jj * P : (jj + 1) * P],
                                rhs=xT[b][:, kt, :],
                                start=(kt == 0),
                                stop=(kt == KO - 1),
                            )
                        qt = qkp.tile([P, RB], BF16, tag="qk")
                        nc.scalar.activation(
                            qt, psq, mybir.ActivationFunctionType.Identity,
                            bias=bq_sb[:, j : j + 1],
                        )
                        qts.append(qt)
                        psk = ps_big.tile([P, RB], F32, tag="ps_big")
                        for kt in range(KO):
                            nc.tensor.matmul(
                                psk,
                                lhsT=wk_sb[:, kt, jj * P : (jj + 1) * P],
                                rhs=xT[b][:, kt, :],
                                start=(kt == 0),
                                stop=(kt == KO - 1),
                            )
                        ktile = qkp.tile([P, RB], BF16, tag="qk")
                        nc.scalar.activation(
                            ktile, psk, mybir.ActivationFunctionType.Identity,
                            bias=bk_sb[:, j : j + 1],
                        )
                        kts.append(ktile)
                    if pending_s is not None:
                        emit_s(*pending_s)
                    pending_s = (c, b, qts, kts)
            if pending_s is not None:
                emit_s(*pending_s)
                pending_s = None

            # --- softmax + transpose of one weight tile ---
            def emit_softmax(i):
                tmask = soft.tile([P, P], F32, tag="tmask")
                nc.vector.tensor_add(tmask, S_all[:, i, :], mask_sb)
                e = soft.tile([P, P], F32, tag="e")
                ssum = soft.tile([P, 1], F32, tag="ssum")
                nc.scalar.activation(
                    e, tmask, mybir.ActivationFunctionType.Exp,
                    scale=float(SCALE), accum_out=ssum,
                )
                rcp = soft.tile([P, 1], F32, tag="rcp")
                nc.vector.reciprocal(rcp, ssum)
                wsb = soft.tile([P, P], BF16, tag="wsb")
                nc.vector.tensor_scalar_mul(wsb, e, rcp)
                pst = ps_t.tile([P, P], BF16, tag="tr")
                nc.tensor.transpose(pst, wsb, ident_sb)
                nc.vector.tensor_copy(wT_all[:, i, :], pst)

            # --- pass 2: V GEMM + O = w @ V + bv ---
            # softmax for tile i is interleaved after the c=0 V chain for i,
            # so the PE streams V matmuls while DVE/ACT run the softmax.
            pending_o = None  # (v_sb, b, rs, c)

            def emit_o(v_sb, b, rs, c):
                i = b * NSUB + rs
                pso = ps_big.tile([P, C2], F32, tag="ps_big")
                nc.tensor.matmul(
                    pso, lhsT=wT_all[:, i, :], rhs=v_sb, start=True, stop=True
                )
                o_sb = opool.tile([P, C2], F32, tag="o")
                nc.vector.tensor_add(o_sb, pso, bv_sb[:, c * C2 : (c + 1) * C2])
                r0 = b * RB + rs * P
                nc.sync.dma_start(out[r0 : r0 + P, c * C2 : (c + 1) * C2], o_sb)

            for c in range(NC2):
                if c + 1 < NC2 and (c + 1) not in wv_tiles:
                    wv_tiles[c + 1] = load_w_chunk(wv_ap, c + 1, C2, "wv")
                wv_sb = wv_tiles.pop(c)
                for b in range(NB):
                    for rs in range(NSUB):
                        psv = ps_big.tile([P, C2], F32, tag="ps_big")
                        for kt in range(KO):
                            nc.tensor.matmul(
                                psv,
                                lhsT=xT[b][:, kt, rs * P : (rs + 1) * P],
                                rhs=wv_sb[:, kt, :],
                                start=(kt == 0),
                                stop=(kt == KO - 1),
                            )
                        v_sb = vpool.tile([P, C2], BF16, tag="v")
                        nc.vector.tensor_copy(v_sb, psv)
                        if c == 0:
                            emit_softmax(b * NSUB + rs)
                        if pending_o is not None:
                            emit_o(*pending_o)
                        pending_o = (v_sb, b, rs, c)
            if pending_o is not None:
                emit_o(*pending_o)
                pending_o = None

    nc.compile()
    return nc


_CACHED = {}


def host_constants():
    mask = np.full((P, P), -1e9, dtype=np.float32)
    for g in range(P // H):
        mask[g * H : (g + 1) * H, g * H : (g + 1) * H] = 0.0
    identity = np.eye(P, dtype=ml_dtypes.bfloat16)
    return mask, identity


def kernel(x, Wq, bq, Wk, bk, Wv, bv):
    x = np.ascontiguousarray(np.asarray(x, dtype=np.float32))
    Wq = np.ascontiguousarray(np.asarray(Wq, dtype=np.float32))
    Wk = np.ascontiguousarray(np.asarray(Wk, dtype=np.float32))
    Wv = np.ascontiguousarray(np.asarray(Wv, dtype=np.float32))
    bq = np.asarray(bq, dtype=np.float32)
    bk = np.asarray(bk, dtype=np.float32)
    bv = np.asarray(bv, dtype=np.float32)

    if "nc" not in _CACHED:
        _CACHED["nc"] = build_program()
    nc = _CACHED["nc"]

    mask, identity = host_constants()
    bqt = np.ascontiguousarray(bq.reshape(KO, P).T)
    bkt = np.ascontiguousarray(bk.reshape(KO, P).T)
    bvr = np.ascontiguousarray(np.broadcast_to(bv, (P, D)))

    in_maps = []
    for i in range(N_CORES):
        in_maps.append(
            {
                "xs": x[i * R : (i + 1) * R],
                "Wq": Wq, "Wk": Wk, "Wv": Wv,
                "bqt": bqt, "bkt": bkt, "bvr": bvr,
                "maskt": mask, "ident": identity,
            }
        )
    res = run_bass_kernel_spmd(nc, in_maps, list(range(N_CORES)))
    return np.concatenate([res.results[i]["out"] for i in range(N_CORES)], axis=0)
